# revision 3
# baseline (speedup 1.0000x reference)
"""JPEG layer (nn_JpegLayer) Trainium2 Bass kernel, 8-core data parallel.

Pipeline per image (per core: 4 images of [3,512,512]):
  P1: 3-accum matmuls fold RGB->YCC color mix + H-DCT (+ vertical 2x-pool for
      chroma) ; route-A, out [h'freq, w]
  T1: PE transposes -> [w, h'freq]
  P2: W-DCT (+ horizontal pool fold for chroma) + DC level-shift correction
      via an extra accumulated rank-structured matmul -> coeffs [w'', h']
  Q : e = d*(1/q); round via +/-2^23*1.5 trick; dec = r*q   (DVE/GPSIMD)
  P3: W-IDCT (+ horizontal 2x upsample fold for chroma) -> [w, h']
  T2: PE transposes -> [h', w]
  P4: H-IDCT (+ vertical upsample fold for chroma) + YCC->RGB fold via
      accumulated matmuls + LEVEL plane via ones-matmul -> psum RGB
  out: DVE tensor_scalar (max 0, min 1) psum->sbuf, DMA out.

All matmul data is float32r (TRN2 reduced-precision fp32 path, 1 cyc/row at
N>=256). Forward-path rounding error ~1e-4 rel; set FP32_FWD=True to run the
forward passes in full fp32 (4 cyc/row) if more accuracy is needed.
"""
import os
import sys
sys.path.insert(0, '/opt/trn_rl_repo')
import numpy as np
import concourse.bacc as bacc
import concourse.bass as bass
import concourse.mybir as mybir
import concourse.tile as tile
from concourse import bass_utils

N_CORES = 8
IMG_PER_CORE = 4
H = W = 512
HT = H // 128            # 4 h-tiles per plane
LEVEL = np.float32(128.0 / 255.0)
LEVEL_F = float(LEVEL)
C_ROUND = 12582912.0   # 1.5*2^23: (x+C)-C == round-half-even(x)
F32 = mybir.dt.float32
F32R = mybir.dt.float32r

RGB2YCC = np.array([[0.299, 0.587, 0.114],
                    [-0.168735892, -0.331264108, 0.5],
                    [0.5, -0.418687589, -0.081312411]], dtype=np.float32)
# YCC2RGB columns: Y col = [1,1,1]; cb col = [0,-0.344136286,1.772]; cr col = [1.402,-0.714136286,0]
CB_C = np.array([0.0, -0.344136286, 1.772], dtype=np.float32)
CR_C = np.array([1.402, -0.714136286, 0.0], dtype=np.float32)


def _dct8():
    i = np.arange(8)[:, None].astype(np.float64)
    j = np.arange(8)[None, :].astype(np.float64)
    m = np.sqrt(2.0 / 8) * np.cos(np.pi * (2 * j + 1) * i / 16.0)
    m[0, :] = 1.0 / np.sqrt(8.0)
    return m.astype(np.float32)


def _blockdiag(b, reps):
    r, c = b.shape
    out = np.zeros((r * reps, c * reps), dtype=np.float32)
    for k in range(reps):
        out[k * r:(k + 1) * r, k * c:(k + 1) * c] = b
    return out


def _build_consts(quantize):
    D = _dct8()
    BD_T = _blockdiag(D.T, 16)             # [128,128] fwd 1D-DCT as lhsT
    BD = _blockdiag(D, 16)                 # [128,128] inverse
    # pooled fwd: PF[16b+2ii+dh, 8b+u] = D[u,ii]/2    [128, 64]
    pf8 = np.zeros((16, 8), dtype=np.float32)
    for ii in range(8):
        for dh in range(2):
            pf8[2 * ii + dh, :] = D[:, ii] * 0.5
    PF = _blockdiag(pf8, 8)                # [128, 64]
    # upsample inverse: PU[8b+v, 16b+2jj+dw] = D[v,jj]   [64, 128]
    pu8 = np.zeros((8, 16), dtype=np.float32)
    for jj in range(8):
        for dw in range(2):
            pu8[:, 2 * jj + dw] = D[jj, :]     # D.T[v,jj] = D[jj,v]? no:
    # careful: idct y[j] = sum_v D[v,j] z[v]  => PU[v, col(j,dw)] = D[v, j]
    pu8 = np.zeros((8, 16), dtype=np.float32)
    for jj in range(8):
        for dw in range(2):
            pu8[:, 2 * jj + dw] = D[:, jj]
    PU = _blockdiag(pu8, 8)                # [64, 128]

    consts = {}
    for c in range(3):
        consts[f"w1y{c}"] = RGB2YCC[0, c] * BD_T
        consts[f"w1c{c}"] = np.concatenate(
            [RGB2YCC[1, c] * PF, RGB2YCC[2, c] * PF], axis=1)  # [128,128]
    consts["w2y"] = BD_T
    consts["w2c"] = PF                     # [128, 64]
    consts["w3y"] = BD
    consts["w3c"] = PU                     # [64, 128]
    consts["w4y"] = BD
    w4 = {}
    for name, cb, cr in (("R", CB_C[0], CR_C[0]), ("G", CB_C[1], CR_C[1]),
                         ("B", CB_C[2], CR_C[2])):
        m = np.zeros((128, 128), dtype=np.float32)
        m[0:64, :] = cb * PU
        m[64:128, :] = cr * PU
        consts[f"w4c{name}"] = m
    consts["ident"] = np.eye(128, dtype=np.float32)

    # quant tables: q = round(quantize[0]*255)/255 (f32, all channels)
    q = (np.round(quantize[0].astype(np.float32) * np.float32(255.0))
         / np.float32(255.0)).astype(np.float32)
    rq = (1.0 / q.astype(np.float64)).astype(np.float32)
    consts["rqt"] = np.tile(rq.T, (16, 64)).astype(np.float32)   # [128,512]
    consts["qt"] = np.tile(q.T, (16, 64)).astype(np.float32)
    # DC correction: coeff d_true = d - 8L*delta00. Via accumulated matmul:
    # lhsT dccor [128,128]: col p (p%8==0) = -8L/128 ; rhs pat8 [128,512]:
    # pat8[k, n] = 1 if n%8==0 else 0  -> psum[p,n] += -8L*d(p%8=0)*d(n%8=0)
    dccor = np.zeros((128, 128), dtype=np.float32)
    dccor[:, 0::8] = np.float32(-8.0 * LEVEL / 128.0)
    consts["dccor"] = dccor
    pat8 = np.zeros((128, 512), dtype=np.float32)
    pat8[:, 0::8] = 1.0
    consts["pat8"] = pat8
    # LEVEL plane: lhsT lones [128,128] all L/128, rhs ones [128,512]
    consts["lones"] = np.full((128, 128), LEVEL / np.float32(128.0),
                              dtype=np.float32)
    consts["ones"] = np.ones((128, 512), dtype=np.float32)
    return consts


_CONST_SHAPES = None


def _build_nc():
    nc = bacc.Bacc("TRN2", target_bir_lowering=False, debug=False,
                   enable_asserts=False, num_devices=N_CORES)
    x_d = nc.dram_tensor("x", [IMG_PER_CORE, 3, H, W], F32R,
                         kind="ExternalInput").ap()
    out_d = nc.dram_tensor("out", [IMG_PER_CORE, 3, H, W], F32,
                           kind="ExternalOutput").ap()
    cd = {}
    for name, shape in _CONST_SHAPES.items():
        cd[name] = nc.dram_tensor(name, list(shape), F32R,
                                  kind="ExternalInput").ap()

    with tile.TileContext(nc) as tc:
        with tc.tile_pool(name="consts", bufs=1) as cp, \
             tc.tile_pool(name="xin", bufs=14) as xp, \
             tc.tile_pool(name="work", bufs=5) as wp, \
             tc.tile_pool(name="stage", bufs=4) as sp, \
             tc.tile_pool(name="psmm", bufs=2, space="PSUM") as pmm, \
             tc.tile_pool(name="pstp", bufs=2, space="PSUM") as ptp:

            cs = {}
            for name, shape in _CONST_SHAPES.items():
                cs[name] = cp.tile(list(shape), F32R, tag=f"c_{name}", name=f"c_{name}")
                nc.sync.dma_start(cs[name][:], cd[name])

            ACT = mybir.ActivationFunctionType
            OP = mybir.AluOpType

            for img in range(IMG_PER_CORE):
                # ---- load RGB tiles ----
                X = {}
                for c in range(3):
                    for t in range(HT):
                        xt = xp.tile([128, 512], F32R, tag="x", name=f"x_{img}_{c}_{t}")
                        nc.sync.dma_start(
                            xt[:], x_d[img, c, 128 * t:128 * (t + 1), :])
                        X[c, t] = xt

                # ---- P1: color + H-DCT (+v-pool chroma) ----
                d1y, d1c = [], []
                for t in range(HT):
                    psY = pmm.tile([128, 512], F32, tag="mm", name="psmm_t")
                    for c in range(3):
                        nc.tensor.matmul(psY[:], cs[f"w1y{c}"][:], X[c, t][:],
                                         start=(c == 0), stop=(c == 2))
                    ty = wp.tile([128, 512], F32R, tag="d1y", name=f"d1y_{img}_{t}")
                    nc.scalar.activation(ty[:], psY[:], ACT.Copy)
                    d1y.append(ty)
                    psC = pmm.tile([128, 512], F32, tag="mm", name="psmm_t")
                    for c in range(3):
                        nc.tensor.matmul(psC[:], cs[f"w1c{c}"][:], X[c, t][:],
                                         start=(c == 0), stop=(c == 2))
                    tcc = wp.tile([128, 512], F32R, tag="d1c", name=f"d1c_{img}_{t}")
                    nc.vector.tensor_copy(tcc[:], psC[:])
                    d1c.append(tcc)

                # ---- T1 ----
                t1y, t1c = [], []
                for s in range(4):
                    pty = ptp.tile([128, 512], F32R, tag="tp", name="pstp_t")
                    for t in range(HT):
                        nc.tensor.transpose(
                            pty[:, 128 * t:128 * (t + 1)],
                            d1y[t][:, 128 * s:128 * (s + 1)], cs["ident"][:])
                    sy = wp.tile([128, 512], F32R, tag="t1y", name=f"t1y_{img}_{s}")
                    nc.scalar.activation(sy[:], pty[:], ACT.Copy)
                    t1y.append(sy)
                    ptc = ptp.tile([128, 512], F32R, tag="tp", name="pstp_t")
                    for t in range(HT):
                        nc.tensor.transpose(
                            ptc[:, 128 * t:128 * (t + 1)],
                            d1c[t][:, 128 * s:128 * (s + 1)], cs["ident"][:])
                    sc = wp.tile([128, 512], F32R, tag="t1c", name=f"t1c_{img}_{s}")
                    nc.vector.tensor_copy(sc[:], ptc[:])
                    t1c.append(sc)

                # ---- P2 + quantize ----
                decy, decc = [], []
                for s in range(4):
                    ps = pmm.tile([128, 512], F32, tag="mm", name="psmm_t")
                    nc.tensor.matmul(ps[:], cs["w2y"][:], t1y[s][:],
                                     start=True, stop=False)
                    nc.tensor.matmul(ps[:], cs["dccor"][:], cs["pat8"][:],
                                     start=False, stop=True)
                    ey = wp.tile([128, 512], F32R, tag="ey", name=f"ey_{img}_{s}")
                    nc.vector.tensor_tensor(ey[:], ps[:], cs["rqt"][:], OP.mult)
                    nc.gpsimd.tensor_scalar(ey[:], ey[:], C_ROUND, C_ROUND,
                                            OP.add, OP.subtract)
                    dy = wp.tile([128, 512], F32R, tag="decy", name=f"decy_{img}_{s}")
                    nc.vector.tensor_tensor(dy[:], ey[:], cs["qt"][:], OP.mult)
                    decy.append(dy)

                    psc = pmm.tile([64, 512], F32, tag="mmc", name="psmmc_t")
                    nc.tensor.matmul(psc[:], cs["w2c"][:], t1c[s][:],
                                     start=True, stop=True)
                    ec = wp.tile([64, 512], F32R, tag="ec", name=f"ec_{img}_{s}")
                    nc.vector.tensor_tensor(ec[:], psc[:], cs["rqt"][0:64, :],
                                            OP.mult)
                    nc.gpsimd.tensor_scalar(ec[:], ec[:], C_ROUND, C_ROUND,
                                            OP.add, OP.subtract)
                    dc = wp.tile([64, 512], F32R, tag="decc", name=f"decc_{img}_{s}")
                    nc.vector.tensor_tensor(dc[:], ec[:], cs["qt"][0:64, :],
                                            OP.mult)
                    decc.append(dc)

                # ---- P3 ----
                p3y, p3c = [], []
                for s in range(4):
                    ps = pmm.tile([128, 512], F32, tag="mm", name="psmm_t")
                    nc.tensor.matmul(ps[:], cs["w3y"][:], decy[s][:],
                                     start=True, stop=True)
                    vy = wp.tile([128, 512], F32R, tag="p3y", name=f"p3y_{img}_{s}")
                    nc.scalar.activation(vy[:], ps[:], ACT.Copy)
                    p3y.append(vy)
                    psc = pmm.tile([128, 512], F32, tag="mm", name="psmm_t")
                    nc.tensor.matmul(psc[:], cs["w3c"][:], decc[s][:],
                                     start=True, stop=True)
                    vc = wp.tile([128, 512], F32R, tag="p3c", name=f"p3c_{img}_{s}")
                    nc.scalar.activation(vc[:], psc[:], ACT.Copy)
                    p3c.append(vc)

                # ---- T2 ----
                t2y, t2c = [], []
                for t in range(4):
                    pty = ptp.tile([128, 512], F32R, tag="tp", name="pstp_t")
                    for s in range(4):
                        nc.tensor.transpose(
                            pty[:, 128 * s:128 * (s + 1)],
                            p3y[s][:, 128 * t:128 * (t + 1)], cs["ident"][:])
                    sy = wp.tile([128, 512], F32R, tag="t2y", name=f"t2y_{img}_{t}")
                    nc.scalar.activation(sy[:], pty[:], ACT.Copy)
                    t2y.append(sy)
                    ptc = ptp.tile([128, 512], F32R, tag="tp", name="pstp_t")
                    for s in range(4):
                        nc.tensor.transpose(
                            ptc[:, 128 * s:128 * (s + 1)],
                            p3c[s][:, 128 * t:128 * (t + 1)], cs["ident"][:])
                    sc = wp.tile([128, 512], F32R, tag="t2c", name=f"t2c_{img}_{t}")
                    nc.vector.tensor_copy(sc[:], ptc[:])
                    t2c.append(sc)

                # ---- P4 + color back + LEVEL + clamp + store ----
                for t in range(4):
                    for ci, cname in enumerate(("R", "G", "B")):
                        ps = pmm.tile([128, 512], F32, tag="mm", name="psmm_t")
                        nc.tensor.matmul(ps[:], cs["w4y"][:], t2y[t][:],
                                         start=True, stop=False)
                        nc.tensor.matmul(ps[:], cs[f"w4c{cname}"][:], t2c[t][:],
                                         start=False, stop=False)
                        nc.tensor.matmul(ps[:], cs["lones"][:], cs["ones"][:],
                                         start=False, stop=True)
                        og = sp.tile([128, 512], F32, tag="og", name=f"og_{img}_{t}_{ci}")
                        nc.vector.tensor_scalar(og[:], ps[:], 0.0, 1.0,
                                                OP.max, OP.min)
                        nc.sync.dma_start(
                            out_d[img, ci, 128 * t:128 * (t + 1), :], og[:])
    nc.compile()
    return nc


_NC_CACHE = None


def kernel(input, quantize):
    global _NC_CACHE, _CONST_SHAPES
    input = np.asarray(input, dtype=np.float32)
    quantize = np.asarray(quantize, dtype=np.float32)
    consts = _build_consts(quantize)
    if _CONST_SHAPES is None:
        _CONST_SHAPES = {k: v.shape for k, v in consts.items()}
    if _NC_CACHE is None:
        _NC_CACHE = _build_nc()
    nc = _NC_CACHE

    in_maps = []
    for core in range(N_CORES):
        shard = np.ascontiguousarray(
            input[core * IMG_PER_CORE:(core + 1) * IMG_PER_CORE])
        m = {"x": shard}
        m.update(consts)
        in_maps.append(m)
    trace = bool(os.environ.get("JPEG_TRACE"))
    kw = {}
    if trace:
        kw["trace"] = True
        td = os.environ.get("JPEG_TRACE_DIR")
        if td:
            os.makedirs(td, exist_ok=True)
            kw["tmpdir"] = td
    res = bass_utils.run_bass_kernel_spmd(nc, in_maps,
                                          core_ids=list(range(N_CORES)), **kw)
    global LAST_RESULT
    LAST_RESULT = res
    out = np.concatenate([res.results[i]["out"] for i in range(N_CORES)],
                         axis=0)
    return out.astype(np.float32)


LAST_RESULT = None



# revision 8
# speedup vs baseline: 2.1140x; 2.1140x over previous
"""JPEG layer (nn_JpegLayer) Trainium2 Bass kernel, 8-core data parallel.

v2 pipeline per image (per core: 4 images of [3,512,512]):
  P1: 3-accum f32r matmuls fold RGB->YCC color mix + H-DCT (+ vertical
      2x-pool for chroma); outputs into [128,1024] 2-bank PSUM pairs.
  T1: PE transposes (f32r, identity rhs) -> [w, h-freq] pairs.
  P2: W-DCT (f32r) + DC level-shift correction via accumulated
      rank-structured matmul (Y only); chroma packed into a single
      [128,1024] pair via partition-offset matmuls (tile_position).
  Q : all on DVE over [128,1024] pairs: e = d*(1/q) (TT, psum read);
      r = (e + 1.5*2^23) - 1.5*2^23 (dual-op tensor_scalar, bf16 out --
      |r| < 256 so bf16 is exact); dec = r*q (bf16 TT).
  S3: fused W-IDCT + transpose as regular bf16 matmuls with dec chunks
      as the stationary operand (replaces P3 matmuls + T2 transposes).
      Chroma 2x horizontal upsample folded into the streamed matrix.
  t2y drain adds sqrt(8)*L on h-freq DC rows (replaces the lones/ones
      LEVEL-plane matmuls); t2c drain is a plain cast.
  P4: bf16 matmuls: H-IDCT + YCC->RGB fold (+ vertical upsample for
      chroma); +L comes in via the t2y DC rows.
  out: clamp [0,1] via DVE dual-op tensor_scalar (B channel split as
      ACT Relu + DVE min to balance engines), DMA out.

Forward path (P1..Q input) stays f32r; only post-round data is bf16.
"""
import os
import sys
sys.path.insert(0, '/opt/trn_rl_repo')
import numpy as np
import ml_dtypes
import concourse.bacc as bacc
import concourse.bass as bass
import concourse.mybir as mybir
import concourse.tile as tile
from concourse import bass_utils

N_CORES = 8
IMG_PER_CORE = 4
H = W = 512
LEVEL = np.float32(128.0 / 255.0)
C_ROUND = 12582912.0   # 1.5*2^23: (x+C)-C == round-half-even(x)
F32 = mybir.dt.float32
F32R = mybir.dt.float32r
BF16 = mybir.dt.bfloat16

RGB2YCC = np.array([[0.299, 0.587, 0.114],
                    [-0.168735892, -0.331264108, 0.5],
                    [0.5, -0.418687589, -0.081312411]], dtype=np.float32)
CB_C = np.array([0.0, -0.344136286, 1.772], dtype=np.float32)
CR_C = np.array([1.402, -0.714136286, 0.0], dtype=np.float32)


def _dct8():
    i = np.arange(8)[:, None].astype(np.float64)
    j = np.arange(8)[None, :].astype(np.float64)
    m = np.sqrt(2.0 / 8) * np.cos(np.pi * (2 * j + 1) * i / 16.0)
    m[0, :] = 1.0 / np.sqrt(8.0)
    return m.astype(np.float32)


def _blockdiag(b, reps):
    r, c = b.shape
    out = np.zeros((r * reps, c * reps), dtype=np.float32)
    for k in range(reps):
        out[k * r:(k + 1) * r, k * c:(k + 1) * c] = b
    return out


def _build_consts(quantize):
    D = _dct8()
    BD_T = _blockdiag(D.T, 16)             # [128,128] fwd 1D-DCT as lhsT
    BD = _blockdiag(D, 16)                 # [128,128] inverse
    # pooled fwd: PF[2ii+dh, u] = D[u,ii]/2 per 16->8 block   [128, 64]
    pf8 = np.zeros((16, 8), dtype=np.float32)
    for ii in range(8):
        for dh in range(2):
            pf8[2 * ii + dh, :] = D[:, ii] * 0.5
    PF = _blockdiag(pf8, 8)                # [128, 64]
    # upsample inverse: pu8[v, 2jj+dw] = D[v,jj]   [64, 128]
    pu8 = np.zeros((8, 16), dtype=np.float32)
    for jj in range(8):
        for dw in range(2):
            pu8[:, 2 * jj + dw] = D[:, jj]
    PU = _blockdiag(pu8, 8)                # [64, 128]

    bf = ml_dtypes.bfloat16
    consts = {}
    for c in range(3):
        consts[f"w1y{c}"] = RGB2YCC[0, c] * BD_T
        consts[f"w1c{c}"] = np.concatenate(
            [RGB2YCC[1, c] * PF, RGB2YCC[2, c] * PF], axis=1)  # [128,128]
    consts["w2y"] = BD_T
    # chroma W-DCT halves: s-even fills psum partitions 0-63, s-odd 64-127,
    # via zero-padded weights accumulated into the same full-partition region
    # (the ISA rejects matmuls with a nonzero dst partition offset).
    w2c_lo = np.zeros((128, 128), dtype=np.float32)
    w2c_lo[:, 0:64] = PF
    w2c_hi = np.zeros((128, 128), dtype=np.float32)
    w2c_hi[:, 64:128] = PF
    consts["w2c_lo"] = w2c_lo
    consts["w2c_hi"] = w2c_hi
    consts["ident"] = np.eye(128, dtype=np.float32)
    # DC correction (Y only): d_true = d - 8L at (w-freq DC, h-freq DC)
    dccor = np.zeros((128, 128), dtype=np.float32)
    dccor[:, 0::8] = np.float32(-8.0 * LEVEL / 128.0)
    consts["dccor"] = dccor
    pat8 = np.zeros((128, 512), dtype=np.float32)
    pat8[:, 0::8] = 1.0
    consts["pat8"] = pat8

    # quant tables over [128,1024] pairs; q = round(quantize[0]*255)/255
    q = (np.round(quantize[0].astype(np.float32) * np.float32(255.0))
         / np.float32(255.0)).astype(np.float32)
    rq = (1.0 / q.astype(np.float64)).astype(np.float32)
    consts["rqt2"] = np.tile(rq.T, (16, 128)).astype(np.float32)  # [128,1024]
    consts["qt2b"] = np.tile(q.T, (16, 128)).astype(bf)           # [128,1024]

    # +L plane: add sqrt(8)*L on h-freq DC rows of t2y during its drain
    lpat = np.zeros((128, 1024), dtype=np.float32)
    lpat[0::8, :] = np.float32(np.sqrt(8.0) * LEVEL)
    consts["lpat"] = lpat

    consts["bdw_b"] = BD.astype(bf)        # S3-Y streamed matrix
    # S3-C streamed matrices: full-K contraction with the inactive 64-row
    # half zeroed (decc packs s-even w2-freqs in rows 0-63, s-odd in 64-127)
    pud_lo = np.zeros((128, 128), dtype=np.float32)
    pud_lo[0:64, :] = PU
    pud_hi = np.zeros((128, 128), dtype=np.float32)
    pud_hi[64:128, :] = PU
    consts["pud_lo"] = pud_lo.astype(bf)
    consts["pud_hi"] = pud_hi.astype(bf)
    consts["w4y_b"] = BD.astype(bf)
    for name, cb, cr in (("R", CB_C[0], CR_C[0]), ("G", CB_C[1], CR_C[1]),
                         ("B", CB_C[2], CR_C[2])):
        m = np.zeros((128, 128), dtype=np.float32)
        m[0:64, :] = cb * PU
        m[64:128, :] = cr * PU
        consts[f"w4c{name}_b"] = m.astype(bf)
    return consts


_CONST_INFO = None


def _build_nc():
    nc = bacc.Bacc("TRN2", target_bir_lowering=False, debug=False,
                   enable_asserts=False, num_devices=N_CORES)
    x_d = nc.dram_tensor("x", [IMG_PER_CORE, 3, H, W], F32R,
                         kind="ExternalInput").ap()
    out_d = nc.dram_tensor("out", [IMG_PER_CORE, 3, H, W], F32,
                           kind="ExternalOutput").ap()
    cd = {}
    for name, (shape, dt) in _CONST_INFO.items():
        cd[name] = nc.dram_tensor(name, list(shape), dt,
                                  kind="ExternalInput").ap()

    ACT = mybir.ActivationFunctionType
    OP = mybir.AluOpType

    with tile.TileContext(nc) as tc:
        with tc.tile_pool(name="consts", bufs=1) as cp, \
             tc.tile_pool(name="xin", bufs=18) as xp, \
             tc.tile_pool(name="fwd", bufs=3) as fp, \
             tc.tile_pool(name="qnt", bufs=4) as qp, \
             tc.tile_pool(name="dcd", bufs=3) as dp, \
             tc.tile_pool(name="outp", bufs=8) as op_, \
             tc.tile_pool(name="psmm", bufs=2, space="PSUM") as pmm, \
             tc.tile_pool(name="pstp", bufs=2, space="PSUM") as ptp:

            cs = {}
            for name, (shape, dt) in _CONST_INFO.items():
                cs[name] = cp.tile(list(shape), dt, tag=f"c_{name}",
                                   name=f"c_{name}")
                nc.sync.dma_start(cs[name][:], cd[name])

            for img in range(IMG_PER_CORE):
                # ---- load RGB tiles ----
                X = {}
                for c in range(3):
                    for t in range(4):
                        xt = xp.tile([128, 512], F32R, tag="x",
                                     name=f"x_{img}_{c}_{t}")
                        nc.sync.dma_start(
                            xt[:], x_d[img, c, 128 * t:128 * (t + 1), :])
                        X[c, t] = xt

                # ---- P1: color + H-DCT (+v-pool chroma), pairs over t ----
                d1y, d1c = [], []
                for j in range(2):
                    psY = pmm.tile([128, 1024], F32, tag="mm", name="psmm")
                    for b in range(2):
                        t = 2 * j + b
                        for c in range(3):
                            nc.tensor.matmul(psY[:, 512 * b:512 * (b + 1)],
                                             cs[f"w1y{c}"][:], X[c, t][:],
                                             start=(c == 0), stop=(c == 2))
                    ty = fp.tile([128, 1024], F32R, tag="d1y",
                                 name=f"d1y_{img}_{j}")
                    nc.scalar.activation(ty[:], psY[:], ACT.Copy)
                    d1y.append(ty)
                for j in range(2):
                    psC = pmm.tile([128, 1024], F32, tag="mm", name="psmm")
                    for b in range(2):
                        t = 2 * j + b
                        for c in range(3):
                            nc.tensor.matmul(psC[:, 512 * b:512 * (b + 1)],
                                             cs[f"w1c{c}"][:], X[c, t][:],
                                             start=(c == 0), stop=(c == 2))
                    tcc = fp.tile([128, 1024], F32R, tag="d1c",
                                  name=f"d1c_{img}_{j}")
                    nc.scalar.activation(tcc[:], psC[:], ACT.Copy)
                    d1c.append(tcc)

                # ---- T1: PE transposes, pairs over s ----
                t1y, t1c = [], []
                for u in range(2):
                    pty = ptp.tile([128, 1024], F32R, tag="tp", name="pstp")
                    for b in range(2):
                        s = 2 * u + b
                        for t in range(4):
                            nc.tensor.transpose(
                                pty[:, 512 * b + 128 * t:512 * b + 128 * (t + 1)],
                                d1y[t // 2][:, 512 * (t % 2) + 128 * s:
                                            512 * (t % 2) + 128 * (s + 1)],
                                cs["ident"][:])
                    sy = fp.tile([128, 1024], F32R, tag="t1y",
                                 name=f"t1y_{img}_{u}")
                    nc.scalar.activation(sy[:], pty[:], ACT.Copy)
                    t1y.append(sy)
                for u in range(2):
                    ptc = ptp.tile([128, 1024], F32R, tag="tp", name="pstp")
                    for b in range(2):
                        s = 2 * u + b
                        for t in range(4):
                            nc.tensor.transpose(
                                ptc[:, 512 * b + 128 * t:512 * b + 128 * (t + 1)],
                                d1c[t // 2][:, 512 * (t % 2) + 128 * s:
                                            512 * (t % 2) + 128 * (s + 1)],
                                cs["ident"][:])
                    sc = fp.tile([128, 1024], F32R, tag="t1c",
                                 name=f"t1c_{img}_{u}")
                    nc.scalar.activation(sc[:], ptc[:], ACT.Copy)
                    t1c.append(sc)

                # ---- P2 + quantize (all DVE) ----
                decy = []
                for u in range(2):
                    ps = pmm.tile([128, 1024], F32, tag="mm", name="psmm")
                    for b in range(2):
                        nc.tensor.matmul(ps[:, 512 * b:512 * (b + 1)],
                                         cs["w2y"][:],
                                         t1y[u][:, 512 * b:512 * (b + 1)],
                                         start=True, stop=False)
                        nc.tensor.matmul(ps[:, 512 * b:512 * (b + 1)],
                                         cs["dccor"][:], cs["pat8"][:],
                                         start=False, stop=True)
                    ey = qp.tile([128, 1024], F32, tag="ey",
                                 name=f"ey_{img}_{u}")
                    nc.vector.tensor_tensor(ey[:], ps[:], cs["rqt2"][:],
                                            OP.mult)
                    ry = qp.tile([128, 1024], BF16, tag="ry",
                                 name=f"ry_{img}_{u}")
                    nc.vector.tensor_scalar(ry[:], ey[:], C_ROUND, C_ROUND,
                                            OP.add, OP.subtract)
                    dy = dp.tile([128, 1024], BF16, tag="decy",
                                 name=f"decy_{img}_{u}")
                    nc.vector.tensor_tensor(dy[:], ry[:], cs["qt2b"][:],
                                            OP.mult)
                    decy.append(dy)

                psc = pmm.tile([128, 1024], F32, tag="mm", name="psmm")
                for s in range(4):
                    nc.tensor.matmul(
                        psc[:, 512 * (s // 2):512 * (s // 2) + 512],
                        cs["w2c_hi" if s % 2 else "w2c_lo"][:],
                        t1c[s // 2][:, 512 * (s % 2):512 * (s % 2) + 512],
                        start=(s % 2 == 0), stop=(s % 2 == 1))
                ec = qp.tile([128, 1024], F32, tag="ey", name=f"ec_{img}")
                nc.vector.tensor_tensor(ec[:], psc[:], cs["rqt2"][:], OP.mult)
                rc = qp.tile([128, 1024], BF16, tag="ry", name=f"rc_{img}")
                nc.vector.tensor_scalar(rc[:], ec[:], C_ROUND, C_ROUND,
                                        OP.add, OP.subtract)
                decc = dp.tile([128, 1024], BF16, tag="decc",
                               name=f"decc_{img}")
                nc.vector.tensor_tensor(decc[:], rc[:], cs["qt2b"][:],
                                        OP.mult)

                # ---- S3: fused W-IDCT + transpose (bf16 matmuls) ----
                t2y, t2c = [], []
                for v in range(2):
                    ps = ptp.tile([128, 1024], F32, tag="tp", name="pstp")
                    for b in range(2):
                        t = 2 * v + b
                        for s in range(4):
                            nc.tensor.matmul(
                                ps[:, 512 * b + 128 * s:512 * b + 128 * (s + 1)],
                                decy[s // 2][:, 512 * (s % 2) + 128 * t:
                                             512 * (s % 2) + 128 * (t + 1)],
                                cs["bdw_b"][:], start=True, stop=True)
                    sy = dp.tile([128, 1024], BF16, tag="t2y",
                                 name=f"t2y_{img}_{v}")
                    nc.vector.tensor_tensor(sy[:], ps[:], cs["lpat"][:],
                                            OP.add)
                    t2y.append(sy)
                for v in range(2):
                    ps = ptp.tile([128, 1024], F32, tag="tp", name="pstp")
                    for b in range(2):
                        t = 2 * v + b
                        for s in range(4):
                            nc.tensor.matmul(
                                ps[:, 512 * b + 128 * s:512 * b + 128 * (s + 1)],
                                decc[:, 512 * (s // 2) + 128 * t:
                                     512 * (s // 2) + 128 * (t + 1)],
                                cs["pud_hi" if s % 2 else "pud_lo"][:],
                                start=True, stop=True)
                    sc = dp.tile([128, 1024], BF16, tag="t2c",
                                 name=f"t2c_{img}_{v}")
                    nc.scalar.activation(sc[:], ps[:], ACT.Copy)
                    t2c.append(sc)

                # ---- P4: H-IDCT + color + clamp + store ----
                for ci, cname in enumerate(("R", "G", "B")):
                    for v in range(2):
                        ps = pmm.tile([128, 1024], F32, tag="mm", name="psmm")
                        for b in range(2):
                            nc.tensor.matmul(
                                ps[:, 512 * b:512 * (b + 1)], cs["w4y_b"][:],
                                t2y[v][:, 512 * b:512 * (b + 1)],
                                start=True, stop=False)
                            nc.tensor.matmul(
                                ps[:, 512 * b:512 * (b + 1)],
                                cs[f"w4c{cname}_b"][:],
                                t2c[v][:, 512 * b:512 * (b + 1)],
                                start=False, stop=True)
                        og = op_.tile([128, 1024], F32, tag="og",
                                      name=f"og_{img}_{ci}_{v}")
                        if ci == 2:
                            nc.scalar.activation(og[:], ps[:], ACT.Relu)
                            nc.vector.tensor_scalar(og[:], og[:], 1.0, None,
                                                    OP.min)
                        else:
                            nc.vector.tensor_scalar(og[:], ps[:], 0.0, 1.0,
                                                    OP.max, OP.min)
                        for b in range(2):
                            t = 2 * v + b
                            nc.sync.dma_start(
                                out_d[img, ci, 128 * t:128 * (t + 1), :],
                                og[:, 512 * b:512 * (b + 1)])
    nc.compile()
    return nc


_NC_CACHE = None


def kernel(input, quantize):
    global _NC_CACHE, _CONST_INFO
    input = np.asarray(input, dtype=np.float32)
    quantize = np.asarray(quantize, dtype=np.float32)
    consts = _build_consts(quantize)
    if _CONST_INFO is None:
        _CONST_INFO = {}
        for k, v in consts.items():
            dt = BF16 if v.dtype == ml_dtypes.bfloat16 else (
                F32 if k in ("rqt2", "lpat") else F32R)
            _CONST_INFO[k] = (v.shape, dt)
    if _NC_CACHE is None:
        _NC_CACHE = _build_nc()
    nc = _NC_CACHE

    in_maps = []
    for core in range(N_CORES):
        shard = np.ascontiguousarray(
            input[core * IMG_PER_CORE:(core + 1) * IMG_PER_CORE])
        m = {"x": shard}
        m.update(consts)
        in_maps.append(m)
    trace = bool(os.environ.get("JPEG_TRACE"))
    kw = {}
    if trace:
        kw["trace"] = True
        td = os.environ.get("JPEG_TRACE_DIR")
        if td:
            os.makedirs(td, exist_ok=True)
            kw["tmpdir"] = td
    res = bass_utils.run_bass_kernel_spmd(nc, in_maps,
                                          core_ids=list(range(N_CORES)), **kw)
    global LAST_RESULT
    LAST_RESULT = res
    out = np.concatenate([res.results[i]["out"] for i in range(N_CORES)],
                         axis=0)
    return out.astype(np.float32)


LAST_RESULT = None


# revision 20
# speedup vs baseline: 2.7249x; 1.2890x over previous
"""JPEG layer (nn_JpegLayer) Trainium2 Bass kernel, 8-core data parallel.

v2 pipeline per image (per core: 4 images of [3,512,512]):
  P1: 3-accum f32r matmuls fold RGB->YCC color mix + H-DCT (+ vertical
      2x-pool for chroma); outputs into [128,1024] 2-bank PSUM pairs.
  T1: PE transposes (f32r, identity rhs) -> [w, h-freq] pairs.
  P2: W-DCT (f32r) + DC level-shift correction via accumulated
      rank-structured matmul (Y only); chroma packed into a single
      [128,1024] pair via partition-offset matmuls (tile_position).
  Q : all on DVE over [128,1024] pairs: e = d*(1/q) (TT, psum read);
      r = (e + 1.5*2^23) - 1.5*2^23 (dual-op tensor_scalar, bf16 out --
      |r| < 256 so bf16 is exact); dec = r*q (bf16 TT).
  S3: fused W-IDCT + transpose as regular bf16 matmuls with dec chunks
      as the stationary operand (replaces P3 matmuls + T2 transposes).
      Chroma 2x horizontal upsample folded into the streamed matrix.
  t2y drain adds sqrt(8)*L on h-freq DC rows (replaces the lones/ones
      LEVEL-plane matmuls); t2c drain is a plain cast.
  P4: bf16 matmuls: H-IDCT + YCC->RGB fold (+ vertical upsample for
      chroma); +L comes in via the t2y DC rows.
  out: clamp [0,1] via DVE dual-op tensor_scalar (B channel split as
      ACT Relu + DVE min to balance engines), DMA out.

Forward path (P1..Q input) stays f32r; only post-round data is bf16.
"""
import os
import sys
sys.path.insert(0, '/opt/trn_rl_repo')
import numpy as np
import ml_dtypes
import concourse.bacc as bacc
import concourse.bass as bass
import concourse.mybir as mybir
import concourse.tile as tile
from concourse import bass_utils

N_CORES = 8
IMG_PER_CORE = 4
H = W = 512
LEVEL = np.float32(128.0 / 255.0)
C_ROUND = 12582912.0   # 1.5*2^23: (x+C)-C == round-half-even(x)
F32 = mybir.dt.float32
F32R = mybir.dt.float32r
BF16 = mybir.dt.bfloat16

RGB2YCC = np.array([[0.299, 0.587, 0.114],
                    [-0.168735892, -0.331264108, 0.5],
                    [0.5, -0.418687589, -0.081312411]], dtype=np.float32)
CB_C = np.array([0.0, -0.344136286, 1.772], dtype=np.float32)
CR_C = np.array([1.402, -0.714136286, 0.0], dtype=np.float32)


def _dct8():
    i = np.arange(8)[:, None].astype(np.float64)
    j = np.arange(8)[None, :].astype(np.float64)
    m = np.sqrt(2.0 / 8) * np.cos(np.pi * (2 * j + 1) * i / 16.0)
    m[0, :] = 1.0 / np.sqrt(8.0)
    return m.astype(np.float32)


def _blockdiag(b, reps):
    r, c = b.shape
    out = np.zeros((r * reps, c * reps), dtype=np.float32)
    for k in range(reps):
        out[k * r:(k + 1) * r, k * c:(k + 1) * c] = b
    return out


def _build_consts(quantize):
    D = _dct8()
    BD_T = _blockdiag(D.T, 16)             # [128,128] fwd 1D-DCT as lhsT
    BD = _blockdiag(D, 16)                 # [128,128] inverse
    # pooled fwd: PF[2ii+dh, u] = D[u,ii]/2 per 16->8 block   [128, 64]
    pf8 = np.zeros((16, 8), dtype=np.float32)
    for ii in range(8):
        for dh in range(2):
            pf8[2 * ii + dh, :] = D[:, ii] * 0.5
    PF = _blockdiag(pf8, 8)                # [128, 64]
    # upsample inverse: pu8[v, 2jj+dw] = D[v,jj]   [64, 128]
    pu8 = np.zeros((8, 16), dtype=np.float32)
    for jj in range(8):
        for dw in range(2):
            pu8[:, 2 * jj + dw] = D[:, jj]
    PU = _blockdiag(pu8, 8)                # [64, 128]

    bf = ml_dtypes.bfloat16
    consts = {}
    for c in range(3):
        consts[f"w1y{c}"] = RGB2YCC[0, c] * BD_T
        consts[f"w1c{c}"] = np.concatenate(
            [RGB2YCC[1, c] * PF, RGB2YCC[2, c] * PF], axis=1)  # [128,128]
    consts["w2y"] = BD_T
    # chroma W-DCT halves: s-even fills psum partitions 0-63, s-odd 64-127,
    # via zero-padded weights accumulated into the same full-partition region
    # (the ISA rejects matmuls with a nonzero dst partition offset).
    w2c_lo = np.zeros((128, 128), dtype=np.float32)
    w2c_lo[:, 0:64] = PF
    w2c_hi = np.zeros((128, 128), dtype=np.float32)
    w2c_hi[:, 64:128] = PF
    consts["w2c_lo"] = w2c_lo
    consts["w2c_hi"] = w2c_hi
    consts["ident"] = np.eye(128, dtype=np.float32)
    # Y level shift as per-partition ACT biases on h-freq DC rows:
    # -sqrt(8)*L on the d1y drain (-L before the DCT pair), +sqrt(8)*L on
    # the t2y drain (+L after the IDCT pair).
    lneg = np.zeros((128, 1), dtype=np.float32)
    lneg[0::8, 0] = -np.float32(np.sqrt(8.0) * LEVEL)
    consts["lneg"] = lneg
    lpos = np.zeros((128, 1), dtype=np.float32)
    lpos[0::8, 0] = np.float32(np.sqrt(8.0) * LEVEL)
    consts["lpos"] = lpos

    # quant tables over [128,1024] pairs; q = round(quantize[0]*255)/255
    q = (np.round(quantize[0].astype(np.float32) * np.float32(255.0))
         / np.float32(255.0)).astype(np.float32)
    rq = (1.0 / q.astype(np.float64)).astype(np.float32)
    consts["rqt2"] = np.tile(rq.T, (16, 128)).astype(np.float32)  # [128,1024]
    consts["qt2b"] = np.tile(q.T, (16, 128)).astype(bf)           # [128,1024]

    consts["bdw_b"] = BD.astype(bf)        # S3-Y streamed matrix
    # S3-C streamed matrices: full-K contraction with the inactive 64-row
    # half zeroed (decc packs s-even w2-freqs in rows 0-63, s-odd in 64-127)
    pud_lo = np.zeros((128, 128), dtype=np.float32)
    pud_lo[0:64, :] = PU
    pud_hi = np.zeros((128, 128), dtype=np.float32)
    pud_hi[64:128, :] = PU
    consts["pud_lo"] = pud_lo.astype(bf)
    consts["pud_hi"] = pud_hi.astype(bf)
    consts["w4y_b"] = BD.astype(bf)
    for name, cb, cr in (("R", CB_C[0], CR_C[0]), ("G", CB_C[1], CR_C[1]),
                         ("B", CB_C[2], CR_C[2])):
        m = np.zeros((128, 128), dtype=np.float32)
        m[0:64, :] = cb * PU
        m[64:128, :] = cr * PU
        consts[f"w4c{name}_b"] = m.astype(bf)
    return consts


_CONST_INFO = None


def _build_nc():
    nc = bacc.Bacc("TRN2", target_bir_lowering=False, debug=False,
                   enable_asserts=False, num_devices=N_CORES)
    x_d = nc.dram_tensor("x", [IMG_PER_CORE, 3, H, W], F32R,
                         kind="ExternalInput").ap()
    out_d = nc.dram_tensor("out", [IMG_PER_CORE, 3, H, W], F32,
                           kind="ExternalOutput").ap()
    cd = {}
    for name, (shape, dt) in _CONST_INFO.items():
        cd[name] = nc.dram_tensor(name, list(shape), dt,
                                  kind="ExternalInput").ap()

    ACT = mybir.ActivationFunctionType
    OP = mybir.AluOpType

    with tile.TileContext(nc) as tc:
        with tc.tile_pool(name="consts", bufs=1) as cp, \
             tc.tile_pool(name="xin", bufs=5) as xp, \
             tc.tile_pool(name="fwd", bufs=3) as fp, \
             tc.tile_pool(name="qnt", bufs=4) as qp, \
             tc.tile_pool(name="dcd", bufs=3) as dp, \
             tc.tile_pool(name="outp", bufs=8) as op_, \
             tc.tile_pool(name="psmm", bufs=2, space="PSUM") as pmm, \
             tc.tile_pool(name="pstp", bufs=2, space="PSUM") as ptp:

            cs = {}
            for name, (shape, dt) in _CONST_INFO.items():
                cs[name] = cp.tile(list(shape), dt, tag=f"c_{name}",
                                   name=f"c_{name}")
                nc.sync.dma_start(cs[name][:], cd[name])

            for img in range(IMG_PER_CORE):
                # ---- load RGB planes (one batched DMA per channel) ----
                X = {}
                for c in range(3):
                    xt = xp.tile([128, 4, 512], F32R, tag="x",
                                 name=f"x_{img}_{c}")
                    nc.sync.dma_start(
                        xt[:],
                        x_d[img, c].rearrange("(t p) w -> p t w", t=4))
                    for t in range(4):
                        X[c, t] = xt[:, t, :]

                # ---- P1: color + H-DCT (+v-pool chroma), pairs over t ----
                d1y, d1c = [], []
                for j in range(2):
                    psY = pmm.tile([128, 1024], F32, tag="mm", name="psmm")
                    for b in range(2):
                        t = 2 * j + b
                        for c in range(3):
                            nc.tensor.matmul(psY[:, 512 * b:512 * (b + 1)],
                                             cs[f"w1y{c}"][:], X[c, t],
                                             start=(c == 0), stop=(c == 2))
                    ty = fp.tile([128, 1024], F32R, tag="d1y",
                                 name=f"d1y_{img}_{j}")
                    nc.scalar.activation(ty[:], psY[:], ACT.Identity,
                                         bias=cs["lneg"][:])
                    d1y.append(ty)
                for j in range(2):
                    psC = pmm.tile([128, 1024], F32, tag="mm", name="psmm")
                    for b in range(2):
                        t = 2 * j + b
                        for c in range(3):
                            nc.tensor.matmul(psC[:, 512 * b:512 * (b + 1)],
                                             cs[f"w1c{c}"][:], X[c, t],
                                             start=(c == 0), stop=(c == 2))
                    tcc = fp.tile([128, 1024], F32R, tag="d1c",
                                  name=f"d1c_{img}_{j}")
                    nc.scalar.activation(tcc[:], psC[:], ACT.Copy)
                    d1c.append(tcc)

                # ---- T1: PE transposes, pairs over s ----
                t1y, t1c = [], []
                for u in range(2):
                    pty = ptp.tile([128, 1024], F32R, tag="tp", name="pstp")
                    for b in range(2):
                        s = 2 * u + b
                        for t in range(4):
                            nc.tensor.transpose(
                                pty[:, 512 * b + 128 * t:512 * b + 128 * (t + 1)],
                                d1y[t // 2][:, 512 * (t % 2) + 128 * s:
                                            512 * (t % 2) + 128 * (s + 1)],
                                cs["ident"][:])
                    sy = fp.tile([128, 1024], F32R, tag="t1y",
                                 name=f"t1y_{img}_{u}")
                    nc.scalar.activation(sy[:], pty[:], ACT.Copy)
                    t1y.append(sy)
                for u in range(2):
                    ptc = ptp.tile([128, 1024], F32R, tag="tp", name="pstp")
                    for b in range(2):
                        s = 2 * u + b
                        for t in range(4):
                            nc.tensor.transpose(
                                ptc[:, 512 * b + 128 * t:512 * b + 128 * (t + 1)],
                                d1c[t // 2][:, 512 * (t % 2) + 128 * s:
                                            512 * (t % 2) + 128 * (s + 1)],
                                cs["ident"][:])
                    sc = fp.tile([128, 1024], F32R, tag="t1c",
                                 name=f"t1c_{img}_{u}")
                    nc.scalar.activation(sc[:], ptc[:], ACT.Copy)
                    t1c.append(sc)

                # ---- P2 + quantize (all DVE) ----
                decy = []
                for u in range(2):
                    ps = pmm.tile([128, 1024], F32, tag="mm", name="psmm")
                    for b in range(2):
                        nc.tensor.matmul(ps[:, 512 * b:512 * (b + 1)],
                                         cs["w2y"][:],
                                         t1y[u][:, 512 * b:512 * (b + 1)],
                                         start=True, stop=True)
                    ey = qp.tile([128, 1024], F32, tag="ey",
                                 name=f"ey_{img}_{u}")
                    nc.vector.tensor_tensor(ey[:], ps[:], cs["rqt2"][:],
                                            OP.mult)
                    ry = qp.tile([128, 1024], BF16, tag="ry",
                                 name=f"ry_{img}_{u}")
                    nc.vector.tensor_scalar(ry[:], ey[:], C_ROUND, C_ROUND,
                                            OP.add, OP.subtract)
                    dy = dp.tile([128, 1024], BF16, tag="decy",
                                 name=f"decy_{img}_{u}")
                    nc.vector.tensor_tensor(dy[:], ry[:], cs["qt2b"][:],
                                            OP.mult)
                    decy.append(dy)

                psc = pmm.tile([128, 1024], F32, tag="mm", name="psmm")
                for s in range(4):
                    nc.tensor.matmul(
                        psc[:, 512 * (s // 2):512 * (s // 2) + 512],
                        cs["w2c_hi" if s % 2 else "w2c_lo"][:],
                        t1c[s // 2][:, 512 * (s % 2):512 * (s % 2) + 512],
                        start=(s % 2 == 0), stop=(s % 2 == 1))
                ec = qp.tile([128, 1024], F32, tag="ey", name=f"ec_{img}")
                nc.vector.tensor_tensor(ec[:], psc[:], cs["rqt2"][:], OP.mult)
                rc = qp.tile([128, 1024], BF16, tag="ry", name=f"rc_{img}")
                nc.vector.tensor_scalar(rc[:], ec[:], C_ROUND, C_ROUND,
                                        OP.add, OP.subtract)
                decc = dp.tile([128, 1024], BF16, tag="decc",
                               name=f"decc_{img}")
                nc.vector.tensor_tensor(decc[:], rc[:], cs["qt2b"][:],
                                        OP.mult)

                # ---- S3: fused W-IDCT + transpose (bf16 matmuls) ----
                t2y, t2c = [], []
                for v in range(2):
                    ps = ptp.tile([128, 1024], F32, tag="tp", name="pstp")
                    for b in range(2):
                        t = 2 * v + b
                        for s in range(4):
                            nc.tensor.matmul(
                                ps[:, 512 * b + 128 * s:512 * b + 128 * (s + 1)],
                                decy[s // 2][:, 512 * (s % 2) + 128 * t:
                                             512 * (s % 2) + 128 * (t + 1)],
                                cs["bdw_b"][:], start=True, stop=True)
                    sy = dp.tile([128, 1024], BF16, tag="t2y",
                                 name=f"t2y_{img}_{v}")
                    nc.scalar.activation(sy[:], ps[:], ACT.Identity,
                                         bias=cs["lpos"][:])
                    t2y.append(sy)
                for v in range(2):
                    ps = ptp.tile([128, 1024], F32, tag="tp", name="pstp")
                    for b in range(2):
                        t = 2 * v + b
                        for s in range(4):
                            nc.tensor.matmul(
                                ps[:, 512 * b + 128 * s:512 * b + 128 * (s + 1)],
                                decc[:, 512 * (s // 2) + 128 * t:
                                     512 * (s // 2) + 128 * (t + 1)],
                                cs["pud_hi" if s % 2 else "pud_lo"][:],
                                start=True, stop=True)
                    sc = dp.tile([128, 1024], BF16, tag="t2c",
                                 name=f"t2c_{img}_{v}")
                    nc.scalar.activation(sc[:], ps[:], ACT.Copy)
                    t2c.append(sc)

                # ---- P4: H-IDCT + color + clamp + store ----
                for ci, cname in enumerate(("R", "G", "B")):
                    for v in range(2):
                        ps = pmm.tile([128, 1024], F32, tag="mm", name="psmm")
                        for b in range(2):
                            nc.tensor.matmul(
                                ps[:, 512 * b:512 * (b + 1)], cs["w4y_b"][:],
                                t2y[v][:, 512 * b:512 * (b + 1)],
                                start=True, stop=False)
                            nc.tensor.matmul(
                                ps[:, 512 * b:512 * (b + 1)],
                                cs[f"w4c{cname}_b"][:],
                                t2c[v][:, 512 * b:512 * (b + 1)],
                                start=False, stop=True)
                        og = op_.tile([128, 1024], F32, tag="og",
                                      name=f"og_{img}_{ci}_{v}")
                        nc.vector.tensor_scalar(og[:], ps[:], 0.0, 1.0,
                                                OP.max, OP.min)
                        nc.sync.dma_start(
                            out_d[img, ci, 256 * v:256 * (v + 1), :]
                            .rearrange("(b p) w -> p b w", b=2),
                            og[:].rearrange("p (b w) -> p b w", b=2))
    nc.compile()
    return nc


_NC_CACHE = None


def kernel(input, quantize):
    global _NC_CACHE, _CONST_INFO
    input = np.asarray(input, dtype=np.float32)
    quantize = np.asarray(quantize, dtype=np.float32)
    consts = _build_consts(quantize)
    if _CONST_INFO is None:
        _CONST_INFO = {}
        for k, v in consts.items():
            dt = BF16 if v.dtype == ml_dtypes.bfloat16 else (
                F32 if k in ("rqt2", "lneg", "lpos") else F32R)
            _CONST_INFO[k] = (v.shape, dt)
    if _NC_CACHE is None:
        _NC_CACHE = _build_nc()
    nc = _NC_CACHE

    in_maps = []
    for core in range(N_CORES):
        shard = np.ascontiguousarray(
            input[core * IMG_PER_CORE:(core + 1) * IMG_PER_CORE])
        m = {"x": shard}
        m.update(consts)
        in_maps.append(m)
    trace = bool(os.environ.get("JPEG_TRACE"))
    kw = {}
    if trace:
        kw["trace"] = True
        td = os.environ.get("JPEG_TRACE_DIR")
        if td:
            os.makedirs(td, exist_ok=True)
            kw["tmpdir"] = td
    res = bass_utils.run_bass_kernel_spmd(nc, in_maps,
                                          core_ids=list(range(N_CORES)), **kw)
    global LAST_RESULT
    LAST_RESULT = res
    out = np.concatenate([res.results[i]["out"] for i in range(N_CORES)],
                         axis=0)
    return out.astype(np.float32)


LAST_RESULT = None
